# revision 30
# baseline (speedup 1.0000x reference)
"""GQA attention block on 8 NeuronCores (Trainium2, Bass/Tile).

Sharding: tensor-parallel over head groups (4 ways: 8 q heads / 2 kv heads
per core) x data-parallel over batch (2 ways).  Each core computes a partial
y = attn_out_slice @ Wo_slice for its (batch, head-group); the host sums the
4 TP partials per batch element.

Host-side prep: x is transposed, cast to bf16, and laid out per-chunk in the
exact SBUF layout (so every DMA partition line is one contiguous 4-16KB run);
weights likewise; q heads are permuted so each head pair shares its kv
head's partition offset.

Per-core device program:
  A) q^T/k^T/v^T projections in bf16 straight from the DMA'd x^T chunks.
  B) attention with two-head software pipelining: heads run in pairs with
     interleaved S -> exp -> PV chains, and PV lags one ki-group behind
     S/exp so the PE never stalls on the exp producers.  exp is split
     across engines: the pair's first head uses the Scalar engine's exp
     (scale 1/sqrt(dh) folded in), the second uses a Schraudolph fast-exp
     on the Vector engine (int16 <- round(A*s + B), bitcast to bf16);
     softmax denominators come from a ones-row in the PV weights;
     normalization = ACT-staged den rows + one batched DVE
     reciprocal_approx_fast per pair + gpsimd partition broadcast + DVE
     multiply.
  C) y = out^T.T @ Wo (bf16), with block tb's 64 matmuls drip-fed 2 per
     inner iteration into block tb+1's attention loop; yp evictions run on
     the Scalar engine and y is written back per 512-column piece in bf16.
"""

import os
import sys

import ml_dtypes
import numpy as np

for _p in ("/opt/trn_rl_repo",):
    if os.path.isdir(_p) and _p not in sys.path:
        sys.path.insert(0, _p)

from contextlib import ExitStack

import concourse.bass as bass  # noqa: F401  (AP types pulled in transitively)
import concourse.mybir as mybir
import concourse.tile as tile
from concourse import bacc
from concourse.bass_utils import run_bass_kernel_spmd
from concourse.masks import make_identity

P = 128
B, T, D = 2, 2048, 2048
HQ, HKV, DH = 32, 8, 64
GROUP = HQ // HKV            # 4
TP = 4                       # tensor-parallel ways
DP = 2                       # data-parallel ways
NCORES = TP * DP
DQ = D // TP                 # 512 q dims per core (8 heads)
DKV = HKV * DH // TP         # 128 kv dims per core (2 kv heads)
NHQ = HQ // TP               # 8 q heads per core
NKV = HKV // TP              # 2 kv heads per core
NKS = D // P                 # 16 contraction subtiles over D
CH = 512                     # T chunk width in projection phase
NCH = T // CH                # 4
TQB = 512                    # T_q block width in attention / psum bank
NTQB = T // TQB              # 4
NKI = T // P                 # 16 key tiles
NB = D // 512                # 4 output column banks
SCALE = 1.0 / 8.0            # 1/sqrt(DH)
F32 = mybir.dt.float32
BF16 = mybir.dt.bfloat16
I16 = mybir.dt.int16
AF = mybir.ActivationFunctionType
# Schraudolph fast-exp in bf16 space: u16 = round(A16*s + B16) bitcast bf16
# ~= exp(s*SCALE) (max rel err ~3.3%); used on the DVE for half the heads.
A16 = 128.0 * 1.4426950408889634 * SCALE
B16 = 16250.5


def _build():
    nc = bacc.Bacc(None, target_bir_lowering=False, debug=False)

    # host-prearranged: every dram tensor is already in SBUF partition-major
    # layout, so each DMA partition line is one contiguous run
    xt_ext = nc.dram_tensor("xt", [NCH, P, NKS * CH], BF16, kind="ExternalInput")
    # wq split per output block so each piece lands just before its q-proj
    wq_ext = nc.dram_tensor("wq", [DQ // P, P, NKS * P], BF16, kind="ExternalInput")
    wk_ext = nc.dram_tensor("wk", [P, NKS * DKV], BF16, kind="ExternalInput")
    wv_ext = nc.dram_tensor("wv", [P, NKS * DKV], BF16, kind="ExternalInput")
    wo_ext = nc.dram_tensor("wo", [P, (DQ // P) * D], BF16, kind="ExternalInput")
    y_ext = nc.dram_tensor("y", [T, D], BF16, kind="ExternalOutput")

    y_v = y_ext[:].rearrange("(to p) n -> p to n", p=P)      # [128,16,2048]

    with tile.TileContext(nc) as tc, ExitStack() as ctx:
        const = ctx.enter_context(tc.tile_pool(name="const", bufs=1))
        xt_p = ctx.enter_context(tc.tile_pool(name="xtp", bufs=8))
        w_p = ctx.enter_context(tc.tile_pool(name="wp", bufs=1))
        row_p = ctx.enter_context(tc.tile_pool(name="rows", bufs=4))
        qt_p = ctx.enter_context(tc.tile_pool(name="qt", bufs=1))
        kt_p = ctx.enter_context(tc.tile_pool(name="kt", bufs=1))
        vo_p = ctx.enter_context(tc.tile_pool(name="vo", bufs=1))
        exp_p = ctx.enter_context(tc.tile_pool(name="expp", bufs=6))
        exq_p = ctx.enter_context(tc.tile_pool(name="exqp", bufs=6))
        bc_p = ctx.enter_context(tc.tile_pool(name="bcp", bufs=2))
        rc_p = ctx.enter_context(tc.tile_pool(name="rcp", bufs=2))
        ot_p = ctx.enter_context(tc.tile_pool(name="otp", bufs=2))

        # PSUM: 8 banks total.
        # s_ps: 2 x [128,1024] (2 banks each) -> 4 banks; attention scores.
        # pv_ps: 2 x [128,512] -> 2 banks; PV accumulators, also phase-A
        #       v-transpose staging.
        # proj_ps: 2 x [128,512] -> 2 banks; projections + Wo outputs.
        s_ps = ctx.enter_context(tc.tile_pool(name="s_ps", bufs=2, space="PSUM"))
        pv_ps = ctx.enter_context(tc.tile_pool(name="pv_ps", bufs=2, space="PSUM"))
        proj_ps = ctx.enter_context(tc.tile_pool(name="proj_ps", bufs=2, space="PSUM"))

        identity = const.tile([P, P], F32)
        make_identity(nc, identity)
        ones_col = const.tile([P, NKV, NKI], F32)
        nc.gpsimd.memset(ones_col[:], 1.0)

        wq_sb = w_p.tile([P, DQ // P, NKS, P], BF16, tag="wq")
        wk_sb = w_p.tile([P, NKS, DKV], BF16, tag="wk")
        wv_sb = w_p.tile([P, NKS, DKV], BF16, tag="wv")
        wo_sb = w_p.tile([P, DQ // P, D], BF16, tag="wo")

        qt_sb = qt_p.tile([P, DQ // P, T], BF16)   # q^T, [dim, t]
        kt_sb = kt_p.tile([P, T], BF16)            # k^T, [dim(2 kv), t]
        vones = vo_p.tile([P, NKV, NKI, DH + 1], BF16)  # [t%128, kv, t//128, dh|1]
        nc.vector.tensor_copy(vones[:, :, :, DH], ones_col[:])

        # preload the exp table set early so the first phase-B exp is cheap
        warm = rc_p.tile([1, 8], F32, tag="rc")
        nc.gpsimd.memset(warm[:], 0.0)
        nc.scalar.activation(warm[:], warm[:], AF.Exp)

        # ---- Phase A: projections from DMA'd x^T chunks ----
        # each chunk arrives as 4 quarter-tiles (4 ks-subtiles each) so the
        # first projection matmuls start after 512KB instead of 2.5MB
        for c in range(NCH):
            xt_qs = []
            for a in range(4):
                xt_q = xt_p.tile([P, 4, CH], BF16, tag="xt", name=f"xtq{a}")
                xt_qs.append(xt_q)
            for a in range(4):
                if c == 0 and a == 0:
                    nc.sync.dma_start(
                        wk_sb[:].rearrange("p ks m -> p (ks m)"), wk_ext[:])
                nc.sync.dma_start(
                    xt_qs[a][:].rearrange("p ks t -> p (ks t)"),
                    xt_ext[c][:, a * 4 * CH:(a + 1) * 4 * CH])
                if c == 0 and a == 2:
                    nc.sync.dma_start(
                        wv_sb[:].rearrange("p ks m -> p (ks m)"), wv_ext[:])
                if c == 0 and a == 3:
                    for mb in range(DQ // P):
                        nc.sync.dma_start(
                            wq_sb[:, mb].rearrange("p ks m -> p (ks m)"),
                            wq_ext[mb])
                    nc.sync.dma_start(
                        wo_sb[:].rearrange("p ks n -> p (ks n)"), wo_ext[:])

            def xt_at(ks):
                return xt_qs[ks // 4][:, ks % 4, :]

            # k^T chunk
            kp = proj_ps.tile([P, CH], F32, tag="proj")
            for ks in range(NKS):
                nc.tensor.matmul(kp[:], wk_sb[:, ks, :],
                                 xt_at(ks),
                                 start=(ks == 0), stop=(ks == NKS - 1))
            nc.scalar.activation(kt_sb[:, c * CH:(c + 1) * CH], kp[:], AF.Copy)
            # v^T chunk, then PE-transpose into vones (v in natural [t, dh] layout)
            vp = proj_ps.tile([P, CH], F32, tag="proj")
            for ks in range(NKS):
                nc.tensor.matmul(vp[:], wv_sb[:, ks, :],
                                 xt_at(ks),
                                 start=(ks == 0), stop=(ks == NKS - 1))
            vt_sb = row_p.tile([P, CH], F32, tag="vt")
            nc.vector.tensor_copy(vt_sb[:], vp[:])
            tpv = pv_ps.tile([P, 4, P], F32, tag="pv")
            for r in range(CH // P):
                nc.tensor.transpose(
                    tpv[:, r, :], vt_sb[:, r * P:(r + 1) * P], identity)
            for j in range(NKV):
                nc.vector.tensor_copy(
                    vones[:, j, c * 4:(c + 1) * 4, 0:DH],
                    tpv[:, :, j * DH:(j + 1) * DH])
            # q^T chunk
            for mb in range(DQ // P):
                qp = proj_ps.tile([P, CH], F32, tag="proj")
                for ks in range(NKS):
                    nc.tensor.matmul(
                        qp[:], wq_sb[:, mb, ks, :],
                        xt_at(ks),
                        start=(ks == 0), stop=(ks == NKS - 1))
                nc.vector.tensor_copy(qt_sb[:, mb, c * CH:(c + 1) * CH], qp[:])

        # ---- Phases B+C interleaved per T_q block ----
        # q heads are permuted host-side to order [0,4,1,5,2,6,3,7] so that
        # head h sits at (block h%4, partition offset 64*(h//4)) -- the
        # partition offset then always equals its kv head's offset in kt_sb,
        # satisfying matmul's equal-base-partition requirement.
        # Heads run in pairs with interleaved S/exp/PV chains; Wo matmuls of
        # the previous T_q block are drip-fed into the PE stream (2 per inner
        # iteration) to fill the gaps left by the exp pipeline.

        def wo_steps(tb):
            """Yield fine-grained phase-C steps for T_q block tb."""
            outt_tb = outt[tb % 2]
            for mi in range(TQB // P):
                mt = tb * (TQB // P) + mi
                y_sb = row_p.tile([P, D], BF16, tag="rows")
                for nb in range(NB):
                    yp = proj_ps.tile([P, 512], F32, tag="proj")
                    for ks in range(DQ // P):
                        yield ("mm", yp, outt_tb, ks, mi, nb)
                    yield ("evict", yp, y_sb, nb, mt)

        def run_wo_step(step):
            kind = step[0]
            if kind == "mm":
                _, yp, outt_tb, ks, mi, nb = step
                nc.tensor.matmul(
                    yp[:], outt_tb[:, ks, mi * P:(mi + 1) * P],
                    wo_sb[:, ks, nb * 512:(nb + 1) * 512],
                    start=(ks == 0), stop=(ks == DQ // P - 1))
            else:
                _, yp, y_sb, nb, mt = step
                nc.vector.tensor_copy(
                    y_sb[:, nb * 512:(nb + 1) * 512], yp[:])
                nc.sync.dma_start(
                    y_v[:, mt, nb * 512:(nb + 1) * 512],
                    y_sb[:, nb * 512:(nb + 1) * 512])

        outt = [None, None]
        pending = []          # phase-C steps of the previous tb
        pending_norm = []     # deferred normalization mults of the previous pair

        def drip_wo(allow_evict, quota=2):
            fed = 0
            while pending and fed < quota:
                if pending[0][0] != "mm" and not allow_evict:
                    break
                step = pending.pop(0)
                run_wo_step(step)
                if step[0] == "mm":
                    fed += 1

        for tb in range(NTQB):
            outt_tb = ot_p.tile([P, DQ // P, TQB], BF16, tag="ot")
            outt[tb % 2] = outt_tb
            for hp in range(NHQ // 2):
                heads = (2 * hp, 2 * hp + 1)
                pvs = [
                    pv_ps.tile([DH + 1, TQB], F32, tag="pv", name=f"pv{i}")
                    for i in range(2)]
                exs = [[None, None] for _ in range(NKI // 2)]

                def emit_pv_chain(i, gp):
                    j = heads[i] // GROUP
                    ex, is_i16 = exs[gp][i]
                    for half in range(2):
                        ki = 2 * gp + half
                        mov = ex[:, half, :]
                        if is_i16:
                            mov = mov.bitcast(BF16)
                        nc.tensor.matmul(
                            pvs[i][:], vones[:, j, ki, :],
                            mov,
                            start=(gp == 0 and half == 0),
                            stop=(gp == NKI // 2 - 1 and half == 1))

                for g in range(NKI // 2):
                    # S + exp: head 0 on ACT, head 1 via DVE Schraudolph
                    # (ACT also covers head 1 for the first two groups so
                    # the DVE can absorb the normalization at boundaries).
                    # Head 1 is emitted first: its sp slot is gated by the
                    # lightly-loaded DVE, giving the ACT-gated head 0 slot
                    # an extra half-group of slack.
                    for i, h in ((1, heads[1]), (0, heads[0])):
                        j = h // GROUP
                        mbq, poq = h % 4, (h // GROUP) * DH
                        sp = s_ps.tile([P, 2, TQB], F32, tag="s")
                        for half in range(2):
                            ki = 2 * g + half
                            nc.tensor.matmul(
                                sp[:, half, :],
                                kt_sb[j * DH:(j + 1) * DH, ki * P:(ki + 1) * P],
                                qt_sb[poq:poq + DH, mbq, tb * TQB:(tb + 1) * TQB],
                                start=True, stop=True)
                        if i == 0 or g < 2:
                            ex = exp_p.tile([P, 2, TQB], BF16, tag="exp")
                            nc.scalar.activation(ex[:], sp[:], AF.Exp, scale=SCALE)
                            exs[g][i] = (ex, False)
                        else:
                            ex = exq_p.tile([P, 2, TQB], I16, tag="exq")
                            nc.vector.tensor_scalar(
                                ex[:], sp[:], A16, B16,
                                mybir.AluOpType.mult, mybir.AluOpType.add)
                            exs[g][i] = (ex, True)
                    # PV runs one group behind S/exp so the PE never
                    # stalls on the exp producers; the first PV is further
                    # delayed to g=2 so it never waits on the previous
                    # pair's normalization to release the pv banks
                    if g == 2:
                        emit_pv_chain(1, 0)
                        emit_pv_chain(1, 1)
                        emit_pv_chain(0, 0)
                        emit_pv_chain(0, 1)
                    elif g > 2:
                        emit_pv_chain(1, g - 1)
                        emit_pv_chain(0, g - 1)
                    drip_wo(allow_evict=True, quota=3 if g < 3 else 2)
                emit_pv_chain(1, NKI // 2 - 1)
                emit_pv_chain(0, NKI // 2 - 1)
                # normalization: den rows staged via ACT (custom-DVE ops
                # read garbage from PSUM on HW), one batched DVE reciprocal
                # and the gpsimd broadcasts now; the two outt mults are
                # deferred into the next pair's g=0/g=1 DVE slots
                den2 = rc_p.tile([1, 2, TQB], F32, tag="den")
                for i in range(2):
                    nc.vector.tensor_copy(den2[:, i, :], pvs[i][DH:DH + 1, :])
                rc2 = rc_p.tile([1, 2, TQB], F32, tag="rc")
                nc.vector.reciprocal_approx_fast(rc2[:], den2[:])
                for i, h in enumerate(heads):
                    mbq, poq = h % 4, (h // GROUP) * DH
                    bc = bc_p.tile([DH, TQB], F32, tag="bc")
                    nc.gpsimd.partition_broadcast(bc[:], rc2[:, i, :],
                                                  channels=DH)
                    nc.vector.tensor_mul(
                        outt_tb[poq:poq + DH, mbq, :],
                        pvs[i][0:DH, :], bc[:])
            # flush any remaining phase-C work of the previous block, then
            # queue this block's
            for step in pending:
                run_wo_step(step)
            pending = list(wo_steps(tb))
        for step in pending:
            run_wo_step(step)

    nc.compile()
    return nc


_NC_CACHE = {}


def _get_nc():
    if "nc" not in _NC_CACHE:
        _NC_CACHE["nc"] = _build()
    return _NC_CACHE["nc"]


def _sbuf_major(w, nks):
    """[nks*P, M] -> [P, nks*M] with row p = concat_ks w[ks*P + p, :]."""
    kpm = np.ascontiguousarray(
        w.reshape(nks, P, -1).transpose(1, 0, 2).reshape(P, -1))
    return kpm


def make_in_maps(x, Wq, Wk, Wv, Wo):
    x = np.asarray(x, dtype=np.float32)
    Wq = np.asarray(Wq, dtype=np.float32)
    Wk = np.asarray(Wk, dtype=np.float32)
    Wv = np.asarray(Wv, dtype=np.float32)
    Wo = np.asarray(Wo, dtype=np.float32)

    # x^T per batch in per-chunk SBUF layout:
    # xt[c, p, ks*CH + t] = x[c*CH + t, ks*P + p]
    xts = []
    for b in range(B):
        xb = x[b].astype(ml_dtypes.bfloat16)               # [T, D]
        a = xb.reshape(NCH, CH, NKS, P).transpose(0, 3, 2, 1)
        xts.append(np.ascontiguousarray(a.reshape(NCH, P, NKS * CH)))
    # interleave the per-core q heads as [0,4,1,5,2,6,3,7] (see phase B note)
    perm = np.concatenate(
        [np.r_[b * DH:(b + 1) * DH, (b + 4) * DH:(b + 5) * DH] for b in range(4)])
    in_maps = []
    for c in range(NCORES):
        b, g = divmod(c, TP)
        wq_c = Wq[:, g * DQ:(g + 1) * DQ][:, perm].astype(ml_dtypes.bfloat16)
        # [D, DQ] -> [mb, P, NKS*P]: piece mb holds q-output cols mb*P:(mb+1)*P
        wq_mb = np.ascontiguousarray(
            wq_c.reshape(NKS, P, DQ // P, P).transpose(2, 1, 0, 3)
            .reshape(DQ // P, P, NKS * P))
        wk_c = Wk[:, g * DKV:(g + 1) * DKV].astype(ml_dtypes.bfloat16)
        wv_c = Wv[:, g * DKV:(g + 1) * DKV].astype(ml_dtypes.bfloat16)
        wo_c = Wo[g * DQ:(g + 1) * DQ, :][perm, :].astype(ml_dtypes.bfloat16)
        in_maps.append({
            "xt": xts[b],
            "wq": wq_mb,
            "wk": _sbuf_major(wk_c, NKS),
            "wv": _sbuf_major(wv_c, NKS),
            "wo": _sbuf_major(wo_c, DQ // P),
        })
    return in_maps


def kernel(x, Wq, Wk, Wv, Wo):
    nc = _get_nc()
    in_maps = make_in_maps(x, Wq, Wk, Wv, Wo)
    res = run_bass_kernel_spmd(nc, in_maps, list(range(NCORES)))
    y = np.zeros((B, T, D), dtype=np.float32)
    for c in range(NCORES):
        b = c // TP
        y[b] += res.results[c]["y"].astype(np.float32)
    return y


# revision 31
# speedup vs baseline: 1.0011x; 1.0011x over previous
"""GQA attention block on 8 NeuronCores (Trainium2, Bass/Tile).

Sharding: tensor-parallel over head groups (4 ways: 8 q heads / 2 kv heads
per core) x data-parallel over batch (2 ways).  Each core computes a partial
y = attn_out_slice @ Wo_slice for its (batch, head-group); the host sums the
4 TP partials per batch element.

Host-side prep: x is transposed, cast to bf16, and laid out per-chunk in the
exact SBUF layout (so every DMA partition line is one contiguous 4-16KB run);
weights likewise; q heads are permuted so each head pair shares its kv
head's partition offset.

Per-core device program:
  A) q^T/k^T/v^T projections in bf16 straight from the DMA'd x^T chunks.
  B) attention with two-head software pipelining: heads run in pairs with
     interleaved S -> exp -> PV chains, and PV lags one ki-group behind
     S/exp so the PE never stalls on the exp producers.  exp is split
     across engines: the pair's first head uses the Scalar engine's exp
     (scale 1/sqrt(dh) folded in), the second uses a Schraudolph fast-exp
     on the Vector engine (int16 <- round(A*s + B), bitcast to bf16);
     softmax denominators come from a ones-row in the PV weights;
     normalization = ACT-staged den rows + one batched DVE
     reciprocal_approx_fast per pair + gpsimd partition broadcast + DVE
     multiply.
  C) y = out^T.T @ Wo (bf16), with block tb's 64 matmuls drip-fed 2 per
     inner iteration into block tb+1's attention loop; yp evictions run on
     the Scalar engine and y is written back per 512-column piece in bf16.
"""

import os
import sys

import ml_dtypes
import numpy as np

for _p in ("/opt/trn_rl_repo",):
    if os.path.isdir(_p) and _p not in sys.path:
        sys.path.insert(0, _p)

from contextlib import ExitStack

import concourse.bass as bass  # noqa: F401  (AP types pulled in transitively)
import concourse.mybir as mybir
import concourse.tile as tile
from concourse import bacc
from concourse.bass_utils import run_bass_kernel_spmd
from concourse.masks import make_identity

P = 128
B, T, D = 2, 2048, 2048
HQ, HKV, DH = 32, 8, 64
GROUP = HQ // HKV            # 4
TP = 4                       # tensor-parallel ways
DP = 2                       # data-parallel ways
NCORES = TP * DP
DQ = D // TP                 # 512 q dims per core (8 heads)
DKV = HKV * DH // TP         # 128 kv dims per core (2 kv heads)
NHQ = HQ // TP               # 8 q heads per core
NKV = HKV // TP              # 2 kv heads per core
NKS = D // P                 # 16 contraction subtiles over D
CH = 512                     # T chunk width in projection phase
NCH = T // CH                # 4
TQB = 512                    # T_q block width in attention / psum bank
NTQB = T // TQB              # 4
NKI = T // P                 # 16 key tiles
NB = D // 512                # 4 output column banks
SCALE = 1.0 / 8.0            # 1/sqrt(DH)
F32 = mybir.dt.float32
BF16 = mybir.dt.bfloat16
I16 = mybir.dt.int16
AF = mybir.ActivationFunctionType
# Schraudolph fast-exp in bf16 space: u16 = round(A16*s + B16) bitcast bf16
# ~= exp(s*SCALE) (max rel err ~3.3%); used on the DVE for half the heads.
A16 = 128.0 * 1.4426950408889634 * SCALE
B16 = 16250.5


def _build():
    nc = bacc.Bacc(None, target_bir_lowering=False, debug=False)

    # host-prearranged: every dram tensor is already in SBUF partition-major
    # layout, so each DMA partition line is one contiguous run
    xt_ext = nc.dram_tensor("xt", [NCH, P, NKS * CH], BF16, kind="ExternalInput")
    # wq split per output block so each piece lands just before its q-proj
    wq_ext = nc.dram_tensor("wq", [DQ // P, P, NKS * P], BF16, kind="ExternalInput")
    wk_ext = nc.dram_tensor("wk", [P, NKS * DKV], BF16, kind="ExternalInput")
    wv_ext = nc.dram_tensor("wv", [P, NKS * DKV], BF16, kind="ExternalInput")
    wo_ext = nc.dram_tensor("wo", [P, (DQ // P) * D], BF16, kind="ExternalInput")
    y_ext = nc.dram_tensor("y", [T, D], BF16, kind="ExternalOutput")

    y_v = y_ext[:].rearrange("(to p) n -> p to n", p=P)      # [128,16,2048]

    with tile.TileContext(nc) as tc, ExitStack() as ctx:
        const = ctx.enter_context(tc.tile_pool(name="const", bufs=1))
        xt_p = ctx.enter_context(tc.tile_pool(name="xtp", bufs=8))
        w_p = ctx.enter_context(tc.tile_pool(name="wp", bufs=1))
        row_p = ctx.enter_context(tc.tile_pool(name="rows", bufs=4))
        qt_p = ctx.enter_context(tc.tile_pool(name="qt", bufs=1))
        kt_p = ctx.enter_context(tc.tile_pool(name="kt", bufs=1))
        vo_p = ctx.enter_context(tc.tile_pool(name="vo", bufs=1))
        exp_p = ctx.enter_context(tc.tile_pool(name="expp", bufs=6))
        exq_p = ctx.enter_context(tc.tile_pool(name="exqp", bufs=6))
        bc_p = ctx.enter_context(tc.tile_pool(name="bcp", bufs=2))
        rc_p = ctx.enter_context(tc.tile_pool(name="rcp", bufs=2))
        ot_p = ctx.enter_context(tc.tile_pool(name="otp", bufs=2))

        # PSUM: 8 banks total.
        # s_ps: 2 x [128,1024] (2 banks each) -> 4 banks; attention scores.
        # pv_ps: 2 x [128,512] -> 2 banks; PV accumulators, also phase-A
        #       v-transpose staging.
        # proj_ps: 2 x [128,512] -> 2 banks; projections + Wo outputs.
        s_ps = ctx.enter_context(tc.tile_pool(name="s_ps", bufs=2, space="PSUM"))
        pv_ps = ctx.enter_context(tc.tile_pool(name="pv_ps", bufs=2, space="PSUM"))
        proj_ps = ctx.enter_context(tc.tile_pool(name="proj_ps", bufs=2, space="PSUM"))

        identity = const.tile([P, P], F32)
        make_identity(nc, identity)
        ones_col = const.tile([P, NKV, NKI], F32)
        nc.gpsimd.memset(ones_col[:], 1.0)

        wq_sb = w_p.tile([P, DQ // P, NKS, P], BF16, tag="wq")
        wk_sb = w_p.tile([P, NKS, DKV], BF16, tag="wk")
        wv_sb = w_p.tile([P, NKS, DKV], BF16, tag="wv")
        wo_sb = w_p.tile([P, DQ // P, D], BF16, tag="wo")

        qt_sb = qt_p.tile([P, DQ // P, T], BF16)   # q^T, [dim, t]
        kt_sb = kt_p.tile([P, T], BF16)            # k^T, [dim(2 kv), t]
        vones = vo_p.tile([P, NKV, NKI, DH + 1], BF16)  # [t%128, kv, t//128, dh|1]
        nc.vector.tensor_copy(vones[:, :, :, DH], ones_col[:])

        # preload the exp table set early so the first phase-B exp is cheap
        warm = rc_p.tile([1, 8], F32, tag="rc")
        nc.gpsimd.memset(warm[:], 0.0)
        nc.scalar.activation(warm[:], warm[:], AF.Exp)

        # ---- Phase A: projections from DMA'd x^T chunks ----
        # each chunk arrives as 4 quarter-tiles (4 ks-subtiles each) so the
        # first projection matmuls start after 512KB instead of 2.5MB
        for c in range(NCH):
            xt_qs = []
            for a in range(4):
                xt_q = xt_p.tile([P, 4, CH], BF16, tag="xt", name=f"xtq{a}")
                xt_qs.append(xt_q)
            for a in range(4):
                if c == 0 and a == 0:
                    nc.sync.dma_start(
                        wk_sb[:].rearrange("p ks m -> p (ks m)"), wk_ext[:])
                nc.sync.dma_start(
                    xt_qs[a][:].rearrange("p ks t -> p (ks t)"),
                    xt_ext[c][:, a * 4 * CH:(a + 1) * 4 * CH])
                if c == 0 and a == 2:
                    nc.sync.dma_start(
                        wv_sb[:].rearrange("p ks m -> p (ks m)"), wv_ext[:])
                if c == 0 and a == 3:
                    for mb in range(DQ // P):
                        nc.sync.dma_start(
                            wq_sb[:, mb].rearrange("p ks m -> p (ks m)"),
                            wq_ext[mb])
                    nc.sync.dma_start(
                        wo_sb[:].rearrange("p ks n -> p (ks n)"), wo_ext[:])

            def xt_at(ks):
                return xt_qs[ks // 4][:, ks % 4, :]

            # k^T chunk
            kp = proj_ps.tile([P, CH], F32, tag="proj")
            for ks in range(NKS):
                nc.tensor.matmul(kp[:], wk_sb[:, ks, :],
                                 xt_at(ks),
                                 start=(ks == 0), stop=(ks == NKS - 1))
            nc.scalar.activation(kt_sb[:, c * CH:(c + 1) * CH], kp[:], AF.Copy)
            # v^T chunk, then PE-transpose into vones (v in natural [t, dh] layout)
            vp = proj_ps.tile([P, CH], F32, tag="proj")
            for ks in range(NKS):
                nc.tensor.matmul(vp[:], wv_sb[:, ks, :],
                                 xt_at(ks),
                                 start=(ks == 0), stop=(ks == NKS - 1))
            vt_sb = row_p.tile([P, CH], F32, tag="vt")
            nc.vector.tensor_copy(vt_sb[:], vp[:])
            tpv = pv_ps.tile([P, 4, P], F32, tag="pv")
            for r in range(CH // P):
                nc.tensor.transpose(
                    tpv[:, r, :], vt_sb[:, r * P:(r + 1) * P], identity)
            for j in range(NKV):
                nc.vector.tensor_copy(
                    vones[:, j, c * 4:(c + 1) * 4, 0:DH],
                    tpv[:, :, j * DH:(j + 1) * DH])
            # q^T chunk
            for mb in range(DQ // P):
                qp = proj_ps.tile([P, CH], F32, tag="proj")
                for ks in range(NKS):
                    nc.tensor.matmul(
                        qp[:], wq_sb[:, mb, ks, :],
                        xt_at(ks),
                        start=(ks == 0), stop=(ks == NKS - 1))
                nc.vector.tensor_copy(qt_sb[:, mb, c * CH:(c + 1) * CH], qp[:])

        # ---- Phases B+C interleaved per T_q block ----
        # q heads are permuted host-side to order [0,4,1,5,2,6,3,7] so that
        # head h sits at (block h%4, partition offset 64*(h//4)) -- the
        # partition offset then always equals its kv head's offset in kt_sb,
        # satisfying matmul's equal-base-partition requirement.
        # Heads run in pairs with interleaved S/exp/PV chains; Wo matmuls of
        # the previous T_q block are drip-fed into the PE stream (2 per inner
        # iteration) to fill the gaps left by the exp pipeline.

        def wo_steps(tb):
            """Yield fine-grained phase-C steps for T_q block tb."""
            outt_tb = outt[tb % 2]
            for mi in range(TQB // P):
                mt = tb * (TQB // P) + mi
                y_sb = row_p.tile([P, D], BF16, tag="rows")
                for nb in range(NB):
                    yp = proj_ps.tile([P, 512], F32, tag="proj")
                    for ks in range(DQ // P):
                        yield ("mm", yp, outt_tb, ks, mi, nb)
                    yield ("evict", yp, y_sb, nb, mt)

        def run_wo_step(step):
            kind = step[0]
            if kind == "mm":
                _, yp, outt_tb, ks, mi, nb = step
                nc.tensor.matmul(
                    yp[:], outt_tb[:, ks, mi * P:(mi + 1) * P],
                    wo_sb[:, ks, nb * 512:(nb + 1) * 512],
                    start=(ks == 0), stop=(ks == DQ // P - 1))
            else:
                _, yp, y_sb, nb, mt = step
                nc.vector.tensor_copy(
                    y_sb[:, nb * 512:(nb + 1) * 512], yp[:])
                nc.sync.dma_start(
                    y_v[:, mt, nb * 512:(nb + 1) * 512],
                    y_sb[:, nb * 512:(nb + 1) * 512])

        outt = [None, None]
        pending = []          # phase-C steps of the previous tb
        pending_norm = []     # deferred normalization mults of the previous pair

        def drip_wo(allow_evict):
            fed = 0
            while pending and fed < 2:
                if pending[0][0] != "mm" and not allow_evict:
                    break
                step = pending.pop(0)
                run_wo_step(step)
                if step[0] == "mm":
                    fed += 1

        for tb in range(NTQB):
            outt_tb = ot_p.tile([P, DQ // P, TQB], BF16, tag="ot")
            outt[tb % 2] = outt_tb
            for hp in range(NHQ // 2):
                heads = (2 * hp, 2 * hp + 1)
                pvs = [
                    pv_ps.tile([DH + 1, TQB], F32, tag="pv", name=f"pv{i}")
                    for i in range(2)]
                exs = [[None, None] for _ in range(NKI // 2)]

                def emit_pv_chain(i, gp):
                    j = heads[i] // GROUP
                    ex, is_i16 = exs[gp][i]
                    for half in range(2):
                        ki = 2 * gp + half
                        mov = ex[:, half, :]
                        if is_i16:
                            mov = mov.bitcast(BF16)
                        nc.tensor.matmul(
                            pvs[i][:], vones[:, j, ki, :],
                            mov,
                            start=(gp == 0 and half == 0),
                            stop=(gp == NKI // 2 - 1 and half == 1))

                for g in range(NKI // 2):
                    # S + exp: head 0 on ACT, head 1 via DVE Schraudolph
                    # (ACT also covers head 1 for the first two groups so
                    # the DVE can absorb the normalization at boundaries).
                    # Head 1 is emitted first: its sp slot is gated by the
                    # lightly-loaded DVE, giving the ACT-gated head 0 slot
                    # an extra half-group of slack.
                    for i, h in ((1, heads[1]), (0, heads[0])):
                        j = h // GROUP
                        mbq, poq = h % 4, (h // GROUP) * DH
                        sp = s_ps.tile([P, 2, TQB], F32, tag="s")
                        for half in range(2):
                            ki = 2 * g + half
                            nc.tensor.matmul(
                                sp[:, half, :],
                                kt_sb[j * DH:(j + 1) * DH, ki * P:(ki + 1) * P],
                                qt_sb[poq:poq + DH, mbq, tb * TQB:(tb + 1) * TQB],
                                start=True, stop=True)
                        if i == 0 or g < 2:
                            ex = exp_p.tile([P, 2, TQB], BF16, tag="exp")
                            nc.scalar.activation(ex[:], sp[:], AF.Exp, scale=SCALE)
                            exs[g][i] = (ex, False)
                        else:
                            ex = exq_p.tile([P, 2, TQB], I16, tag="exq")
                            nc.vector.tensor_scalar(
                                ex[:], sp[:], A16, B16,
                                mybir.AluOpType.mult, mybir.AluOpType.add)
                            exs[g][i] = (ex, True)
                    # PV runs one group behind S/exp so the PE never
                    # stalls on the exp producers; the first PV is further
                    # delayed to g=2 so it never waits on the previous
                    # pair's normalization to release the pv banks
                    if g == 2:
                        emit_pv_chain(1, 0)
                        emit_pv_chain(1, 1)
                        emit_pv_chain(0, 0)
                        emit_pv_chain(0, 1)
                    elif g > 2:
                        emit_pv_chain(1, g - 1)
                        emit_pv_chain(0, g - 1)
                    drip_wo(allow_evict=True)
                emit_pv_chain(1, NKI // 2 - 1)
                emit_pv_chain(0, NKI // 2 - 1)
                # normalization: den rows staged via ACT (custom-DVE ops
                # read garbage from PSUM on HW), one batched DVE reciprocal
                # and the gpsimd broadcasts now; the two outt mults are
                # deferred into the next pair's g=0/g=1 DVE slots
                den2 = rc_p.tile([1, 2, TQB], F32, tag="den")
                for i in range(2):
                    nc.scalar.activation(
                        den2[:, i, :], pvs[i][DH:DH + 1, :], AF.Copy)
                rc2 = rc_p.tile([1, 2, TQB], F32, tag="rc")
                nc.vector.reciprocal_approx_fast(rc2[:], den2[:])
                for i, h in enumerate(heads):
                    mbq, poq = h % 4, (h // GROUP) * DH
                    bc = bc_p.tile([DH, TQB], F32, tag="bc")
                    nc.gpsimd.partition_broadcast(bc[:], rc2[:, i, :],
                                                  channels=DH)
                    nc.vector.tensor_mul(
                        outt_tb[poq:poq + DH, mbq, :],
                        pvs[i][0:DH, :], bc[:])
            # flush any remaining phase-C work of the previous block, then
            # queue this block's
            for step in pending:
                run_wo_step(step)
            pending = list(wo_steps(tb))
        for step in pending:
            run_wo_step(step)

    nc.compile()
    return nc


_NC_CACHE = {}


def _get_nc():
    if "nc" not in _NC_CACHE:
        _NC_CACHE["nc"] = _build()
    return _NC_CACHE["nc"]


def _sbuf_major(w, nks):
    """[nks*P, M] -> [P, nks*M] with row p = concat_ks w[ks*P + p, :]."""
    kpm = np.ascontiguousarray(
        w.reshape(nks, P, -1).transpose(1, 0, 2).reshape(P, -1))
    return kpm


def make_in_maps(x, Wq, Wk, Wv, Wo):
    x = np.asarray(x, dtype=np.float32)
    Wq = np.asarray(Wq, dtype=np.float32)
    Wk = np.asarray(Wk, dtype=np.float32)
    Wv = np.asarray(Wv, dtype=np.float32)
    Wo = np.asarray(Wo, dtype=np.float32)

    # x^T per batch in per-chunk SBUF layout:
    # xt[c, p, ks*CH + t] = x[c*CH + t, ks*P + p]
    xts = []
    for b in range(B):
        xb = x[b].astype(ml_dtypes.bfloat16)               # [T, D]
        a = xb.reshape(NCH, CH, NKS, P).transpose(0, 3, 2, 1)
        xts.append(np.ascontiguousarray(a.reshape(NCH, P, NKS * CH)))
    # interleave the per-core q heads as [0,4,1,5,2,6,3,7] (see phase B note)
    perm = np.concatenate(
        [np.r_[b * DH:(b + 1) * DH, (b + 4) * DH:(b + 5) * DH] for b in range(4)])
    in_maps = []
    for c in range(NCORES):
        b, g = divmod(c, TP)
        wq_c = Wq[:, g * DQ:(g + 1) * DQ][:, perm].astype(ml_dtypes.bfloat16)
        # [D, DQ] -> [mb, P, NKS*P]: piece mb holds q-output cols mb*P:(mb+1)*P
        wq_mb = np.ascontiguousarray(
            wq_c.reshape(NKS, P, DQ // P, P).transpose(2, 1, 0, 3)
            .reshape(DQ // P, P, NKS * P))
        wk_c = Wk[:, g * DKV:(g + 1) * DKV].astype(ml_dtypes.bfloat16)
        wv_c = Wv[:, g * DKV:(g + 1) * DKV].astype(ml_dtypes.bfloat16)
        wo_c = Wo[g * DQ:(g + 1) * DQ, :][perm, :].astype(ml_dtypes.bfloat16)
        in_maps.append({
            "xt": xts[b],
            "wq": wq_mb,
            "wk": _sbuf_major(wk_c, NKS),
            "wv": _sbuf_major(wv_c, NKS),
            "wo": _sbuf_major(wo_c, DQ // P),
        })
    return in_maps


def kernel(x, Wq, Wk, Wv, Wo):
    nc = _get_nc()
    in_maps = make_in_maps(x, Wq, Wk, Wv, Wo)
    res = run_bass_kernel_spmd(nc, in_maps, list(range(NCORES)))
    y = np.zeros((B, T, D), dtype=np.float32)
    for c in range(NCORES):
        b = c // TP
        y[b] += res.results[c]["y"].astype(np.float32)
    return y
